# revision 14
# baseline (speedup 1.0000x reference)
"""DeepSeek-V3 MLA attention wrapper kernel for 8 Trainium2 NeuronCores.

Sharding: core c = (b, qh) with b = c // 2 (batch index), qh = c % 2 (token
half). Each core runs the KV path (down-proj, k_nope, v up-proj) only for its
OWN 1024-token half, then the two cores of a batch exchange the resulting
k_nope/v/k_pe with a 2-core AllGather (HBM->HBM), so the per-batch KV work is
not duplicated. Q path + attention + output projection cover the core's own
1024 query rows; per-core outputs are disjoint row blocks the host concats.

All activations are feature-major ([features_partition, tokens_free]).
Matmuls on the score path, p*v, and the output projection run in bf16
(1 col/cycle on the PE, half the DMA/SBUF bytes); projections feeding
RMSNorms stay float32r. Softmax row sums are accumulated on the DVE in bf16
(2x mode) and reduced over partitions with a single ones-matmul per (head,
query-tile) rather than one per key-block, keeping the PE stream lean.
RoPE de-interleave + rotate-half are folded into augmented weight rows on the
host; RMSNorm ln-weights are folded into the up-projection weights.
"""

import sys

sys.path.insert(0, "/opt/trn_rl_repo")

import numpy as np

import concourse.tile as tile
from concourse import bacc, mybir

B, S, H = 4, 2048, 16
HID = 2048
Q_LORA = 1536
KV_LORA = 512
D_NOPE, D_ROPE, D_V = 128, 64, 128
D_QK = D_NOPE + D_ROPE  # 192
THETA = 10000.0
EPS = 1e-6

P = 128
TQ = S // 2  # tokens per core (queries, and own KV half)
NT = 512  # token tile (matmul free dim)
N_CORES = 8
PAIRS = [[0, 1], [2, 3], [4, 5], [6, 7]]

F32 = mybir.dt.float32
F32R = mybir.dt.float32r
BF16 = mybir.dt.bfloat16
EXP = mybir.ActivationFunctionType.Exp
SQRT = mybir.ActivationFunctionType.Sqrt

KVA_ROWS = KV_LORA + 2 * P  # 512 ckv | [pe(64) pad] | [rot(64) pad]
QB_ROWS = H * 256  # pair j: nope2j | nope2j+1 | [pe2j|pe2j+1] | [rot2j|rot2j+1]

_CACHE = {}


def build_nc():
    import os
    PH = int(os.environ.get("KPHASES", "6"))
    REP = int(os.environ.get("KREPEAT", "1"))
    key = ("nc", PH, REP)
    if key in _CACHE:
        return _CACHE[key]
    nc = bacc.Bacc(None, target_bir_lowering=False, num_devices=N_CORES)

    xq = nc.dram_tensor("xq", [HID, TQ], F32R, kind="ExternalInput")
    wqa = nc.dram_tensor(
        "wqa", [HID // P, Q_LORA // P, P, P], F32R, kind="ExternalInput"
    )
    wqb = nc.dram_tensor(
        "wqb", [Q_LORA // P, QB_ROWS // P, P, P], F32R, kind="ExternalInput"
    )
    wkva = nc.dram_tensor("wkva", [HID, KVA_ROWS], F32R, kind="ExternalInput")
    wnope = nc.dram_tensor(
        "wnope", [KV_LORA // P, H, P, P], F32R, kind="ExternalInput"
    )
    wv = nc.dram_tensor("wv", [KV_LORA, H * D_V], F32R, kind="ExternalInput")
    wo = nc.dram_tensor(
        "wo", [H * D_V // P, HID // P, P, P], BF16, kind="ExternalInput"
    )
    cosq = nc.dram_tensor("cosq", [P, TQ], F32, kind="ExternalInput")
    sinq = nc.dram_tensor("sinq", [P, TQ], F32, kind="ExternalInput")
    cosk = nc.dram_tensor("cosk", [D_ROPE, TQ], F32, kind="ExternalInput")
    sink = nc.dram_tensor("sink", [D_ROPE, TQ], F32, kind="ExternalInput")
    ones_in = nc.dram_tensor("ones", [P, 1], F32R, kind="ExternalInput")
    outT = nc.dram_tensor("outT", [HID, TQ], F32, kind="ExternalOutput")

    # cross-core exchange buffers (HBM); slot 17 rows 0:64 hold k_pe
    knp_loc = nc.dram_tensor("knp_loc", [H + 1, P, TQ], BF16, kind="Internal")
    knp_gth = nc.dram_tensor("knp_gth", [2, H + 1, P, TQ], BF16, kind="Internal")
    v_loc = nc.dram_tensor("v_loc", [TQ // P, P, H * D_V], BF16, kind="Internal")
    v_gth = nc.dram_tensor("v_gth", [2, TQ // P, P, H * D_V], BF16, kind="Internal")

    SCALE = float(D_QK) ** -0.5
    NQT = TQ // NT  # 2 (query tiles; also own-half key tiles)
    NKH = HID // P  # 16
    NKQ = Q_LORA // P  # 12
    NKV = KV_LORA // P  # 4

    with tile.TileContext(nc) as tc:
        with (
            tc.tile_pool(name="const", bufs=1) as const,
            tc.tile_pool(name="dram", bufs=1, space="DRAM") as dram,
        ):
            ones_col = const.tile([P, 1], F32R, name="ones_col")
            nc.sync.dma_start(ones_col, ones_in[:, :])
            ones_row = const.tile([1, P], F32R, name="ones_row")
            nc.sync.dma_start(ones_row, ones_in[:, :].rearrange("p one -> one p"))
            ones_col_bf = const.tile([P, 1], BF16, name="ones_col_bf")
            nc.vector.memset(ones_col_bf, 1.0)
            eps_t = const.tile([1, 1], F32, name="eps_t")
            nc.vector.memset(eps_t, EPS)

            qf_spill = dram.tile([QB_ROWS // P, P, TQ], BF16, name="qf_spill")

            for rep in range(REP):
                with (
                    tc.tile_pool(name=f"x_pool{rep}", bufs=1) as x_pool,
                    tc.tile_pool(name=f"kpe_pool{rep}", bufs=1) as kpe_pool,
                ):
                    # x for this core's token half: used by KV-down AND Q-down
                    xq3 = []
                    for k in range(NKH):
                        xk = x_pool.tile([P, TQ], F32R, name="xk", tag=f"xq{k}")
                        (nc.gpsimd if k % 2 else nc.sync).dma_start(
                            xk, xq[k * P : (k + 1) * P, :]
                        )
                        xq3.append(xk)
                    kpe_sb = kpe_pool.tile([D_ROPE, S], BF16, name="kpe_sb")

                    # ============ P1: KV down-proj (own token half) ==========
                    if PH >= 1:
                        with (
                            tc.tile_pool(name=f"ckv_pool{rep}", bufs=1) as ckv_pool,
                        ):
                            ckv_sb = ckv_pool.tile(
                                [P, NKV, TQ], F32R, name="ckv_sb"
                            )
                            with (
                                tc.tile_pool(name=f"p1s{rep}", bufs=1) as p1s,
                                tc.tile_pool(name=f"p1scr{rep}", bufs=1) as p1scr,
                                tc.tile_pool(name=f"p1ps{rep}", bufs=1, space="PSUM") as p1ps,
                            ):
                                ck = p1s.tile([D_ROPE, TQ], F32, name="ck")
                                sk = p1s.tile([D_ROPE, TQ], F32, name="sk")
                                nc.sync.dma_start(ck, cosk[:, :])
                                nc.sync.dma_start(sk, sink[:, :])
                                kpe_own = p1s.tile([P, TQ], BF16, name="kpe_own")
                                nc.vector.memset(kpe_own[D_ROPE:], 0.0)
                                wkva_sb = []
                                for k in range(NKH):
                                    wk = p1s.tile(
                                        [P, KVA_ROWS], F32R, name="wk",
                                        tag=f"wkva{k}", bufs=1,
                                    )
                                    (nc.gpsimd if k % 2 else nc.sync).dma_start(
                                        wk, wkva[k * P : (k + 1) * P, :]
                                    )
                                    wkva_sb.append(wk)
                                for tt in range(NQT):
                                    tcs = slice(tt * NT, (tt + 1) * NT)
                                    acc_sq1 = p1scr.tile(
                                        [P, NT], F32R, name="acc_sq1",
                                        tag="acc_sq1", bufs=2,
                                    )
                                    for m in [4, 0, 1, 2, 3]:
                                        pt = p1ps.tile(
                                            [P, NT], F32, name="pt", tag="kv_ps",
                                            bufs=3,
                                        )
                                        for k in range(NKH):
                                            nc.tensor.matmul(
                                                pt,
                                                wkva_sb[k][:, m * P : (m + 1) * P],
                                                xq3[k][:, tcs],
                                                start=(k == 0),
                                                stop=(k == NKH - 1),
                                            )
                                        if m == 4:
                                            rot = p1scr.tile(
                                                [D_ROPE, NT], F32, name="rot",
                                                tag="krot", bufs=2,
                                            )
                                            nc.vector.tensor_scalar_mul(
                                                rot[0:32], pt[32:64], -1.0
                                            )
                                            nc.vector.tensor_copy(
                                                rot[32:64], pt[0:32]
                                            )
                                            tmp = p1scr.tile(
                                                [D_ROPE, NT], F32, name="tmp",
                                                tag="ktmp", bufs=2,
                                            )
                                            nc.vector.tensor_mul(
                                                tmp, rot, sk[:, tcs]
                                            )
                                            kc_sb = p1scr.tile(
                                                [D_ROPE, NT], F32, name="kc_sb",
                                                tag="kcos", bufs=2,
                                            )
                                            nc.vector.tensor_mul(
                                                kc_sb, pt[:D_ROPE], ck[:, tcs]
                                            )
                                            nc.vector.tensor_add(
                                                kpe_own[:D_ROPE, tcs], kc_sb, tmp
                                            )
                                        else:
                                            nc.vector.tensor_copy(
                                                ckv_sb[:, m, tcs], pt
                                            )
                                            sq = p1scr.tile(
                                                [P, NT], F32R, name="sq",
                                                tag="sq", bufs=3,
                                            )
                                            nc.scalar.square(sq, pt)
                                            if m == 0:
                                                nc.vector.tensor_copy(acc_sq1, sq)
                                            else:
                                                nc.vector.tensor_add(
                                                    acc_sq1, acc_sq1, sq
                                                )
                                    part = p1ps.tile(
                                        [1, NT], F32, name="part", tag="part",
                                        bufs=2,
                                    )
                                    nc.tensor.matmul(
                                        part, ones_col, acc_sq1,
                                        start=True, stop=True,
                                    )
                                    rms = p1scr.tile(
                                        [1, NT], F32, name="rms", tag="rms", bufs=2
                                    )
                                    nc.scalar.activation(
                                        rms, part, SQRT, bias=eps_t,
                                        scale=1.0 / KV_LORA,
                                    )
                                    inv = p1scr.tile(
                                        [1, NT], F32R, name="inv", tag="inv", bufs=2
                                    )
                                    with nc.allow_low_precision(
                                        reason="f32r is fp32-width"
                                    ):
                                        nc.vector.reciprocal(inv, rms)
                                    bc = p1ps.tile(
                                        [P, NT], F32, name="bc", tag="part", bufs=2
                                    )
                                    nc.tensor.matmul(
                                        bc, ones_row, inv, start=True, stop=True
                                    )
                                    ib = p1scr.tile(
                                        [P, NT], F32R, name="ib", tag="ib", bufs=2
                                    )
                                    nc.vector.tensor_copy(ib, bc)
                                    for m in range(NKV):
                                        sl = ckv_sb[:, m, tcs]
                                        nc.vector.tensor_mul(sl, sl, ib)
                                nc.sync.dma_start(knp_loc[H], kpe_own)

                            # ===== KV up (own half): k_nope + v, then gather =
                            if PH >= 2:
                                with (
                                    tc.tile_pool(name=f"pk{rep}", bufs=1) as pk,
                                    tc.tile_pool(name=f"pkps{rep}", bufs=1, space="PSUM") as pkps,
                                ):
                                    for h in range(H):
                                        wbs = []
                                        for k in range(NKV):
                                            wb = pk.tile(
                                                [P, P], F32R, name="wb",
                                                tag="wn_blk", bufs=8,
                                            )
                                            nc.sync.dma_start(wb, wnope[k, h])
                                            wbs.append(wb)
                                        knh = pk.tile(
                                            [P, TQ], BF16, name="knh", tag="knh",
                                            bufs=2,
                                        )
                                        for tt in range(NQT):
                                            tcs = slice(tt * NT, (tt + 1) * NT)
                                            pt = pkps.tile(
                                                [P, NT], F32, name="pt",
                                                tag="knv", bufs=4,
                                            )
                                            for k in range(NKV):
                                                nc.tensor.matmul(
                                                    pt,
                                                    wbs[k],
                                                    ckv_sb[:, k, tcs],
                                                    start=(k == 0),
                                                    stop=(k == NKV - 1),
                                                )
                                            nc.vector.tensor_copy(knh[:, tcs], pt)
                                        (nc.gpsimd if h % 2 else nc.sync).dma_start(
                                            knp_loc[h], knh
                                        )
                                    NG = H * D_V // NT  # 4 head groups
                                    for g in range(NG):
                                        pans = []
                                        for k in range(NKV):
                                            pan = pk.tile(
                                                [P, NT], F32R, name="pan",
                                                tag="wv_pan", bufs=8,
                                            )
                                            nc.sync.dma_start(
                                                pan,
                                                wv[k * P : (k + 1) * P,
                                                   g * NT : (g + 1) * NT],
                                            )
                                            pans.append(pan)
                                        for ti in range(TQ // P):
                                            pt = pkps.tile(
                                                [P, NT], F32, name="pt",
                                                tag="knv", bufs=4,
                                            )
                                            for k in range(NKV):
                                                nc.tensor.matmul(
                                                    pt,
                                                    ckv_sb[:, k, ti * P : (ti + 1) * P],
                                                    pans[k],
                                                    start=(k == 0),
                                                    stop=(k == NKV - 1),
                                                )
                                            vt = pk.tile(
                                                [P, NT], BF16, name="vt",
                                                tag="vt", bufs=3,
                                            )
                                            nc.vector.tensor_copy(vt, pt)
                                            (nc.gpsimd if ti % 2 else nc.sync).dma_start(
                                                v_loc[ti, :, g * NT : (g + 1) * NT],
                                                vt,
                                            )
                                    nc.gpsimd.collective_compute(
                                        "AllGather",
                                        mybir.AluOpType.bypass,
                                        replica_groups=PAIRS,
                                        ins=[knp_loc[:, :, :].opt()],
                                        outs=[knp_gth[:, :, :, :].opt()],
                                    )
                                    nc.gpsimd.collective_compute(
                                        "AllGather",
                                        mybir.AluOpType.bypass,
                                        replica_groups=PAIRS,
                                        ins=[v_loc[:, :, :].opt()],
                                        outs=[v_gth[:, :, :, :].opt()],
                                    )

                    # ================= P2 + P3: Q path =======================
                    if PH >= 3:
                        with tc.tile_pool(name=f"qa_pool{rep}", bufs=1) as qa_pool:
                            qa_sb = qa_pool.tile([P, NKQ, TQ], F32R, name="qa_sb")
                            with (
                                tc.tile_pool(name=f"p2scr{rep}", bufs=1) as p2scr,
                                tc.tile_pool(name=f"p2ps{rep}", bufs=1, space="PSUM") as p2ps,
                            ):
                                ss_ps = [
                                    p2ps.tile([1, NT], F32, name=f"ss{q}", tag=f"ss{q}")
                                    for q in range(NQT)
                                ]
                                acc_sq = [
                                    p2scr.tile(
                                        [P, NT], F32R, name=f"accsq{q}", tag=f"accsq{q}"
                                    )
                                    for q in range(NQT)
                                ]
                                for m in range(NKQ):
                                    pts = [
                                        p2ps.tile([P, NT], F32, name="pt", tag="qa_ps", bufs=4)
                                        for _ in range(NQT)
                                    ]
                                    for k in range(NKH):
                                        wb = p2scr.tile(
                                            [P, P], F32R, name="wb", tag="wqa_blk", bufs=16
                                        )
                                        (nc.sync if k % 2 else nc.gpsimd).dma_start(wb, wqa[k, m])
                                        for q in range(NQT):
                                            nc.tensor.matmul(
                                                pts[q],
                                                wb,
                                                xq3[k][:, q * NT : (q + 1) * NT],
                                                start=(k == 0),
                                                stop=(k == NKH - 1),
                                            )
                                    for q in range(NQT):
                                        nc.vector.tensor_copy(
                                            qa_sb[:, m, q * NT : (q + 1) * NT], pts[q]
                                        )
                                        sq = p2scr.tile(
                                            [P, NT], F32R, name="sq", tag="sq", bufs=3
                                        )
                                        nc.scalar.square(sq, pts[q])
                                        if m == 0:
                                            nc.vector.tensor_copy(acc_sq[q], sq)
                                        else:
                                            nc.vector.tensor_add(
                                                acc_sq[q], acc_sq[q], sq
                                            )
                                for q in range(NQT):
                                    nc.tensor.matmul(
                                        ss_ps[q], ones_col, acc_sq[q],
                                        start=True, stop=True,
                                    )
                                    rms = p2scr.tile(
                                        [1, NT], F32, name="rms", tag="rms", bufs=2
                                    )
                                    nc.scalar.activation(
                                        rms, ss_ps[q], SQRT, bias=eps_t, scale=1.0 / Q_LORA
                                    )
                                    inv = p2scr.tile(
                                        [1, NT], F32R, name="inv", tag="inv", bufs=2
                                    )
                                    with nc.allow_low_precision(reason="f32r is fp32-width"):
                                        nc.vector.reciprocal(inv, rms)
                                    bc = p2ps.tile([P, NT], F32, name="bc", tag="bc", bufs=1)
                                    nc.tensor.matmul(bc, ones_row, inv, start=True, stop=True)
                                    ib = p2scr.tile([P, NT], F32R, name="ib", tag="ib", bufs=2)
                                    nc.vector.tensor_copy(ib, bc)
                                    for m in range(NKQ):
                                        sl = qa_sb[:, m, q * NT : (q + 1) * NT]
                                        nc.vector.tensor_mul(sl, sl, ib)

                            # ---------------- P3: Q up-proj + rope ---------------
                            if PH >= 4:
                                with (
                                    tc.tile_pool(name=f"p3scr{rep}", bufs=1) as p3scr,
                                    tc.tile_pool(name=f"p3ps{rep}", bufs=1, space="PSUM") as p3ps,
                                ):
                                    cq = p3scr.tile([P, TQ], F32, name="cq")
                                    sq_t = p3scr.tile([P, TQ], F32, name="sq_t")
                                    nc.sync.dma_start(cq, cosq[:, :])
                                    nc.sync.dma_start(sq_t, sinq[:, :])
                                    for j in range(H // 2):
                                        for mi in range(3):
                                            m = 4 * j + mi
                                            pts = [
                                                p3ps.tile(
                                                    [P, NT], F32, name="pt", tag="qf_ps", bufs=6
                                                )
                                                for _ in range(NQT)
                                            ]
                                            for k in range(NKQ):
                                                wb = p3scr.tile(
                                                    [P, P], F32R, name="wb", tag="wqb_blk", bufs=16
                                                )
                                                (nc.sync if k % 2 else nc.gpsimd).dma_start(wb, wqb[k, m])
                                                for q in range(NQT):
                                                    nc.tensor.matmul(
                                                        pts[q],
                                                        wb,
                                                        qa_sb[:, k, q * NT : (q + 1) * NT],
                                                        start=(k == 0),
                                                        stop=(k == NKQ - 1),
                                                    )
                                            if mi < 2:
                                                for q in range(NQT):
                                                    sb = p3scr.tile(
                                                        [P, NT], BF16, name="sb", tag="qf_sb",
                                                        bufs=3,
                                                    )
                                                    nc.vector.tensor_copy(sb, pts[q])
                                                    nc.scalar.dma_start(
                                                        qf_spill[m, :, q * NT : (q + 1) * NT], sb
                                                    )
                                            else:
                                                for q in range(NQT):
                                                    cs = slice(q * NT, (q + 1) * NT)
                                                    rot = p3scr.tile(
                                                        [P, NT], F32, name="rot", tag="rot",
                                                        bufs=2,
                                                    )
                                                    nc.vector.tensor_scalar_mul(
                                                        rot[0:32], pts[q][32:64], -1.0
                                                    )
                                                    nc.vector.tensor_copy(
                                                        rot[32:64], pts[q][0:32]
                                                    )
                                                    nc.vector.tensor_scalar_mul(
                                                        rot[64:96], pts[q][96:128], -1.0
                                                    )
                                                    nc.vector.tensor_copy(
                                                        rot[96:128], pts[q][64:96]
                                                    )
                                                    tmp = p3scr.tile(
                                                        [P, NT], F32, name="tmp", tag="rtmp",
                                                        bufs=2,
                                                    )
                                                    nc.vector.tensor_mul(
                                                        tmp, rot, sq_t[:, cs]
                                                    )
                                                    pe_sb = p3scr.tile(
                                                        [P, NT], BF16, name="pe_sb", tag="pe_sb",
                                                        bufs=3,
                                                    )
                                                    nc.vector.tensor_mul(
                                                        pe_sb, pts[q], cq[:, cs]
                                                    )
                                                    nc.vector.tensor_add(pe_sb, pe_sb, tmp)
                                                    nc.scalar.dma_start(
                                                        qf_spill[4 * j + 2, :, cs], pe_sb
                                                    )

                    # =========== P45: attention over gathered KV =============
                    if PH >= 5:
                        with tc.tile_pool(name=f"ot_pool{rep}", bufs=1) as ot_pool:
                            ot_tiles = [
                                ot_pool.tile(
                                    [P, TQ], BF16, name=f"ot{h}", tag=f"ot{h}", bufs=1
                                )
                                for h in range(H)
                            ]
                            with (
                                tc.tile_pool(name=f"p5s{rep}", bufs=1) as p5s,
                                tc.tile_pool(name=f"p5scr{rep}", bufs=1) as p5scr,
                                tc.tile_pool(name=f"p5ps{rep}", bufs=1, space="PSUM") as p5ps,
                            ):
                                for sl in range(2):
                                    nc.sync.dma_start(
                                        kpe_sb[:, sl * TQ : (sl + 1) * TQ],
                                        knp_gth[sl, H, :D_ROPE, :],
                                    )
                                NG = H * D_V // NT  # 4 head groups
                                for g in range(NG):
                                    v_g = p5s.tile(
                                        [P, S // P, NT], BF16, name="v_g", tag="v_g",
                                        bufs=2,
                                    )
                                    for ti in range(S // P):
                                        sl, t8 = divmod(ti, TQ // P)
                                        (nc.gpsimd if ti % 2 else nc.sync).dma_start(
                                            v_g[:, ti],
                                            v_gth[sl, t8, :, g * NT : (g + 1) * NT],
                                        )
                                    for h in range(4 * g, 4 * g + 4):
                                        hj = h % 4
                                        j, d = h // 2, h % 2
                                        kn_h = p5s.tile(
                                            [P, S], BF16, name="kn_h", tag="kn_h", bufs=2
                                        )
                                        for sl in range(2):
                                            (nc.gpsimd if sl else nc.sync).dma_start(
                                                kn_h[:, sl * TQ : (sl + 1) * TQ],
                                                knp_gth[sl, h],
                                            )
                                        # ---- q loads ----
                                        qn_h = p5s.tile(
                                            [P, TQ], BF16, name="qn_h", tag="qn_h", bufs=2
                                        )
                                        nc.scalar.dma_start(qn_h, qf_spill[4 * j + d])
                                        qpe_h = p5s.tile(
                                            [D_ROPE, TQ], BF16, name="qpe_h", tag="qpe_h",
                                            bufs=2,
                                        )
                                        nc.scalar.dma_start(
                                            qpe_h,
                                            qf_spill[
                                                4 * j + 2, d * D_ROPE : (d + 1) * D_ROPE, :
                                            ],
                                        )
                                        # ---- attention (qt-inner, shared lhsT) ----
                                        pos = {}
                                        prs = {}
                                        accs = {}
                                        for qt in range(NQT):
                                            pos[qt] = p5ps.tile(
                                                [P, NT], F32, name="po", tag="po", bufs=2
                                            )
                                            prs[qt] = p5ps.tile(
                                                [1, NT], F32, name="pr", tag="pr", bufs=2
                                            )
                                            accs[qt] = p5scr.tile(
                                                [P, NT], BF16, name="acc", tag="acc",
                                                bufs=2,
                                            )
                                        for kc in range(S // P):
                                            kcs = slice(kc * P, (kc + 1) * P)
                                            pst = {}
                                            for qt in range(NQT):
                                                pst[qt] = p5ps.tile(
                                                    [P, NT], F32, name="pst", tag="st2",
                                                    bufs=4,
                                                )
                                            for qt in range(NQT):
                                                nc.tensor.matmul(
                                                    pst[qt],
                                                    kn_h[:, kcs],
                                                    qn_h[:, qt * NT : (qt + 1) * NT],
                                                    start=True,
                                                    stop=False,
                                                )
                                            for qt in range(NQT):
                                                nc.tensor.matmul(
                                                    pst[qt],
                                                    kpe_sb[:, kcs],
                                                    qpe_h[:, qt * NT : (qt + 1) * NT],
                                                    start=False,
                                                    stop=True,
                                                )
                                            p_sbs = {}
                                            for qt in range(NQT):
                                                p_sbs[qt] = p5scr.tile(
                                                    [P, NT], BF16, name="p_sb",
                                                    tag="p_sb", bufs=4,
                                                )
                                                nc.scalar.activation(
                                                    p_sbs[qt], pst[qt], EXP, scale=SCALE
                                                )
                                            for qt in range(NQT):
                                                if kc == 0:
                                                    nc.vector.tensor_copy(
                                                        accs[qt], p_sbs[qt]
                                                    )
                                                else:
                                                    nc.vector.tensor_add(
                                                        accs[qt], accs[qt], p_sbs[qt]
                                                    )
                                            for qt in range(NQT):
                                                nc.tensor.matmul(
                                                    pos[qt],
                                                    v_g[:, kc, hj * P : (hj + 1) * P],
                                                    p_sbs[qt],
                                                    start=(kc == 0),
                                                    stop=(kc == S // P - 1),
                                                    skip_group_check=True,
                                                )
                                        for qt in range(NQT):
                                            qcs = slice(qt * NT, (qt + 1) * NT)
                                            nc.tensor.matmul(
                                                prs[qt], ones_col_bf, accs[qt],
                                                start=True, stop=True,
                                            )
                                            inv = p5scr.tile(
                                                [1, NT], F32R, name="inv", tag="inv",
                                                bufs=2,
                                            )
                                            with nc.allow_low_precision(
                                                reason="f32r is fp32-width"
                                            ):
                                                nc.vector.reciprocal(inv, prs[qt])
                                            bc = p5ps.tile(
                                                [P, NT], F32, name="bc", tag="pr", bufs=2
                                            )
                                            nc.tensor.matmul(
                                                bc, ones_row, inv, start=True, stop=True
                                            )
                                            ib = p5scr.tile(
                                                [P, NT], F32R, name="ib", tag="ib", bufs=2
                                            )
                                            nc.vector.tensor_copy(ib, bc)
                                            nc.vector.tensor_mul(
                                                ot_tiles[h][:, qcs], pos[qt], ib
                                            )

                            # ================= P6: output projection =============
                            if PH >= 6:
                                with (
                                    tc.tile_pool(name=f"p6scr{rep}", bufs=1) as p6scr,
                                    tc.tile_pool(name=f"p6ps{rep}", bufs=1, space="PSUM") as p6ps,
                                ):
                                    NKO = H * D_V // P  # 16
                                    for mg in range(HID // P // 2):
                                        pts = {}
                                        for mi in range(2):
                                            for q in range(NQT):
                                                pts[(mi, q)] = p6ps.tile(
                                                    [P, NT], F32, name="pt",
                                                    tag=f"oo{2 * (mg % 2) + mi}", bufs=2,
                                                )
                                        for k in range(NKO):
                                            for mi in range(2):
                                                m = 2 * mg + mi
                                                wb = p6scr.tile(
                                                    [P, P], BF16, name="wb",
                                                    tag="wo_blk", bufs=16,
                                                )
                                                (nc.sync if k % 2 else nc.gpsimd).dma_start(wb, wo[k, m])
                                                for q in range(NQT):
                                                    nc.tensor.matmul(
                                                        pts[(mi, q)],
                                                        wb,
                                                        ot_tiles[k][
                                                            :, q * NT : (q + 1) * NT
                                                        ],
                                                        start=(k == 0),
                                                        stop=(k == NKO - 1),
                                                    )
                                        for mi in range(2):
                                            m = 2 * mg + mi
                                            for q in range(NQT):
                                                sb = p6scr.tile(
                                                    [P, NT], F32, name="sb", tag="o_sb",
                                                    bufs=4,
                                                )
                                                nc.vector.tensor_copy(sb, pts[(mi, q)])
                                                (nc.sync if m % 2 else nc.gpsimd).dma_start(
                                                    outT[
                                                        m * P : (m + 1) * P,
                                                        q * NT : (q + 1) * NT,
                                                    ],
                                                    sb,
                                                )

    nc.compile()
    _CACHE[key] = nc
    return nc


# ======================= host-side preparation ===========================


def _deint_perm(d):
    half = d // 2
    perm = np.empty(d, dtype=np.int64)
    perm[:half] = 2 * np.arange(half)
    perm[half:] = 2 * np.arange(half) + 1
    return perm


def _rot_of(rows):
    half = rows.shape[0] // 2
    return np.concatenate([-rows[half:], rows[:half]], axis=0)


def _block(w_t, kp, mp):
    """[K, M] -> [K//P, M//P, P, P] contiguous blocks."""
    K, M = w_t.shape
    return np.ascontiguousarray(
        w_t.reshape(K // kp, kp, M // mp, mp).transpose(0, 2, 1, 3)
    )


def prepare_host_inputs(x, q_a_w, q_a_ln_w, q_b_w, kv_a_w, kv_a_ln_w, kv_b_w, o_w):
    import ml_dtypes

    perm = _deint_perm(D_ROPE)

    qb = (q_b_w * q_a_ln_w[None, :]).reshape(H, D_QK, Q_LORA)
    qb_aug = np.zeros((QB_ROWS, Q_LORA), dtype=np.float32)
    for j in range(H // 2):
        h0, h1 = 2 * j, 2 * j + 1
        base = j * 512
        qb_aug[base : base + 128] = qb[h0, :D_NOPE]
        qb_aug[base + 128 : base + 256] = qb[h1, :D_NOPE]
        pe0 = qb[h0, D_NOPE:][perm]
        pe1 = qb[h1, D_NOPE:][perm]
        qb_aug[base + 256 : base + 320] = pe0
        qb_aug[base + 320 : base + 384] = pe1
        qb_aug[base + 384 : base + 448] = _rot_of(pe0)
        qb_aug[base + 448 : base + 512] = _rot_of(pe1)

    kva_aug = np.zeros((KVA_ROWS, HID), dtype=np.float32)
    kva_aug[:KV_LORA] = kv_a_w[:KV_LORA]
    pe_rows = kv_a_w[KV_LORA:][perm]
    kva_aug[KV_LORA : KV_LORA + D_ROPE] = pe_rows
    kva_aug[KV_LORA + P : KV_LORA + P + D_ROPE] = _rot_of(pe_rows)

    kvb = (kv_b_w * kv_a_ln_w[None, :]).reshape(H, D_NOPE + D_V, KV_LORA)
    wnope_t = kvb[:, :D_NOPE].reshape(H * D_NOPE, KV_LORA).T  # [KV_LORA, H*128]
    wv_t = np.ascontiguousarray(kvb[:, D_NOPE:].reshape(H * D_V, KV_LORA).T)

    inv_freq = 1.0 / (THETA ** (np.arange(0, D_ROPE, 2, dtype=np.float32) / D_ROPE))
    t = np.arange(S, dtype=np.float32)
    ang = np.outer(inv_freq, t)
    cos = np.concatenate([np.cos(ang)] * 2, axis=0).astype(np.float32)  # [64, S]
    sin = np.concatenate([np.sin(ang)] * 2, axis=0).astype(np.float32)
    cos2 = np.concatenate([cos, cos], axis=0)  # [128, S]
    sin2 = np.concatenate([sin, sin], axis=0)

    shared = {
        "ones": np.ones((P, 1), dtype=np.float32),
        "wqa": _block(np.ascontiguousarray(q_a_w.T), P, P),
        "wqb": _block(np.ascontiguousarray(qb_aug.T), P, P),
        "wkva": np.ascontiguousarray(kva_aug.T),
        "wnope": _block(np.ascontiguousarray(wnope_t), P, P),
        "wv": wv_t,
        "wo": _block(np.ascontiguousarray(o_w.T), P, P).astype(ml_dtypes.bfloat16),
    }
    per_core = []
    for c in range(N_CORES):
        b, qh = c // 2, c % 2
        xTb = np.ascontiguousarray(x[b].T.astype(np.float32))
        qs = qh * TQ
        m = dict(shared)
        m["xq"] = np.ascontiguousarray(xTb[:, qs : qs + TQ])
        m["cosq"] = np.ascontiguousarray(cos2[:, qs : qs + TQ])
        m["sinq"] = np.ascontiguousarray(sin2[:, qs : qs + TQ])
        m["cosk"] = np.ascontiguousarray(cos[:, qs : qs + TQ])
        m["sink"] = np.ascontiguousarray(sin[:, qs : qs + TQ])
        per_core.append(m)
    return per_core


def kernel(x, q_a_w, q_a_ln_w, q_b_w, kv_a_w, kv_a_ln_w, kv_b_w, o_w):
    from concourse.bass_utils import run_bass_kernel_spmd

    nc = build_nc()
    per_core = prepare_host_inputs(
        np.asarray(x),
        np.asarray(q_a_w),
        np.asarray(q_a_ln_w),
        np.asarray(q_b_w),
        np.asarray(kv_a_w),
        np.asarray(kv_a_ln_w),
        np.asarray(kv_b_w),
        np.asarray(o_w),
    )
    res = run_bass_kernel_spmd(nc, per_core, core_ids=list(range(N_CORES)))
    out = np.empty((B, S, HID), dtype=np.float32)
    for c in range(N_CORES):
        b, qh = c // 2, c % 2
        out[b, qh * TQ : (qh + 1) * TQ] = res.results[c]["outT"].T
    return out
